# revision 30
# baseline (speedup 1.0000x reference)
"""Self-contained Trainium2 Bass kernel for nn_EntAttentionLayer.

Sharding: 8 cores = (batch 4) x (sequence half 2), no collectives.
Each core computes its [1024 tokens, 1024 hid] slice of the output
end-to-end: self-attention (banded mask) -> cross-attention to tag
embeddings -> FFN, each with residual + LayerNorm.

Device layout: activations kept transposed [hid(part), tok(free)].
  - the six H x H projections run in fp8e4m3 DoubleRow perf mode
    (2 contraction tiles per instruction); weights staged x16 on host
    (fp8 dynamic range), the 1/16 folded into PSUM evacuations. The
    FFN stays bf16: fp8 noise there lands directly on the output
    (no downstream LN to attenuate it) and blows the error budget.
  - scores computed transposed S^T[k, q] = (K^T slice).T @ (Q^T slice)
    in bf16; softmax exp(s/8 - 2) on ACT, batched over both heads of a
    pair ([128, 2x512] PSUM tile); the -2 shift cancels in the
    normalize and keeps p in range
  - band mask added via identity-matmul accumulation into PSUM (value
    8.0 pre-compensates the 1/8 exp scale)
  - sum_k exp folded into PV matmul via a ones-column appended to V
  - LayerNorm stats via bf16 ones-vector matmuls on [x | x^2] copies
  - [1,N] -> [128,N] broadcasts via gpsimd partition_broadcast (no DMA)
  - schedule: attention runs qn-outer; the qn0 out-proj + LN1 and the
    qn0 cross-attention blocks interleave into qn1's (ACT/exp-bound)
    attention window via per-head hooks
Per-core inputs are staged with the sequence ROTATED by half*1024 so all
8 cores run the identical program (band tiles are core-local data).
Residual/LN math in fp32.
"""

import sys

for _p in ("/opt/trn_rl_repo",):
    if _p not in sys.path:
        sys.path.insert(0, _p)

import numpy as np
import ml_dtypes

import concourse.bacc as bacc
import concourse.mybir as mybir
import concourse.tile as tile
from concourse.tile import add_dep_helper
from concourse.bass_utils import run_bass_kernel_spmd

BF = ml_dtypes.bfloat16
F8 = ml_dtypes.float8_e4m3
fp32 = mybir.dt.float32
bf16 = mybir.dt.bfloat16
f8 = mybir.dt.float8e4
DR = mybir.MatmulPerfMode.DoubleRow
ADD = mybir.AluOpType.add
MULT = mybir.AluOpType.mult

H = 1024          # hidden
S = 2048          # full sequence
QL = 1024         # per-core query tokens
FFN = 4096
NH, HD = 16, 64
P = 128
HT = H // P       # 8 hid tiles
ST = S // P       # 16 seq tiles
QN = QL // 512    # 2 q blocks of 512
EPS = 1e-12
WS = 16.0         # fp8 weight staging scale
IWS = 1.0 / WS

# pvec column offsets (per-partition param pack, [128, PCOLS] fp32)
QB, KB, SLG, SLB = 0, 8, 16, 24
CQB, CKB, COB, CLG, CLB = 32, 40, 48, 56, 64
IB, OB, OLG, OLB = 72, 104, 112, 120
NEG2 = 128  # column of -2.0 (exp shift)
PCOLS = 129

USE_POOL = False   # gpsimd partition_broadcast + swdge DMA
USE_POOL_OPS = False  # gpsimd elementwise ops (slow on HW: software Q7 path)
_CACHE = {}


def _band_needed(er):
    """(kt, qn) pairs, in local (rotated) coords, where the band tile can be
    nonzero for either half. Core-independent."""
    out = []
    for kt in range(ST):
        for qn in range(QN):
            lo, hi = qn * 512 - er, qn * 512 + 511 + er
            k0, k1 = kt * P, kt * P + 127
            if (k0 <= hi and k1 >= lo) or (k0 - S <= hi and k1 - S >= lo):
                out.append((kt, qn))
    return out


def _build(er):
    """Build + bacc-compile the per-core program. er = ent_range (>=0)."""
    band_kq = _band_needed(er) if er > 0 else []
    nc = bacc.Bacc()

    # ---- DRAM I/O ----
    xT = nc.dram_tensor("xT", [H, S], f8, kind="ExternalInput")
    xh = nc.dram_tensor("xh", [H, QL], fp32, kind="ExternalInput")
    w_sq = nc.dram_tensor("w_sq", [H, H], f8, kind="ExternalInput")
    w_sk = nc.dram_tensor("w_sk", [H, H], f8, kind="ExternalInput")
    w_sv = nc.dram_tensor("w_sv", [H, H], f8, kind="ExternalInput")
    w_so = nc.dram_tensor("w_so", [H, H], f8, kind="ExternalInput")
    w_cq = nc.dram_tensor("w_cq", [H, H], f8, kind="ExternalInput")
    w_ck = nc.dram_tensor("w_ck", [H, H], f8, kind="ExternalInput")
    w_cv = nc.dram_tensor("w_cv", [H, H], f8, kind="ExternalInput")
    w_co = nc.dram_tensor("w_co", [H, H], f8, kind="ExternalInput")
    w_i = nc.dram_tensor("w_i", [H, FFN], bf16, kind="ExternalInput")
    w_o = nc.dram_tensor("w_o", [FFN, H], bf16, kind="ExternalInput")
    pvec = nc.dram_tensor("pvec", [P, PCOLS], fp32, kind="ExternalInput")
    tagsT = nc.dram_tensor("tagsT", [H, 64], f8, kind="ExternalInput")
    ident_d = nc.dram_tensor("ident", [P, P], f8, kind="ExternalInput")
    ones_d = nc.dram_tensor("ones", [P, 1], bf16, kind="ExternalInput")
    nb = max(len(band_kq), 1)
    band_d = nc.dram_tensor("band", [nb, P, 512], f8, kind="ExternalInput")
    yT = nc.dram_tensor("yT", [H, QL], fp32, kind="ExternalOutput")

    T = 50  # tags count (tagsT padded to 64 cols)
    FT = FFN // P  # 32
    FC = 8         # FFN m-tiles per chunk (4 chunks)

    with tile.TileContext(nc) as tc:
        with tc.tile_pool(name="p1", bufs=1) as p1, \
             tc.tile_pool(name="p2", bufs=2) as p2, \
             tc.tile_pool(name="p3", bufs=3) as p3, \
             tc.tile_pool(name="p4", bufs=4) as p4, \
             tc.tile_pool(name="psA", bufs=2, space="PSUM") as psA, \
             tc.tile_pool(name="psB", bufs=2, space="PSUM") as psB, \
             tc.tile_pool(name="psC", bufs=2, space="PSUM") as psC, \
             tc.tile_pool(name="dram", bufs=1, space="DRAM") as dpool, \
             tc.tile_pool(name="dscr", bufs=4, space="DRAM") as dscr:

            # ---- constants ----
            identt = p1.tile([P, P], f8, tag="ident")
            nc.sync.dma_start(out=identt[:], in_=ident_d[:, :])
            onest = p1.tile([P, 1], bf16, tag="ones")
            nc.sync.dma_start(out=onest[:], in_=ones_d[:, :])
            pv = p1.tile([P, PCOLS], fp32, tag="pvec")
            nc.sync.dma_start(out=pv[:], in_=pvec[:, :])
            tg = p1.tile([P, HT, 64], f8, tag="tags")
            nc.sync.dma_start(out=tg[:], in_=tagsT[:, :].rearrange("(t p) c -> p t c", p=P))
            eps_t = p1.tile([1, 1], fp32, tag="eps")
            nc.vector.memset(eps_t[:], EPS)

            def col(c, n=P):
                return pv[0:n, c:c + 1]

            def bcast(out_ap, in_ap, channels, name):
                if USE_POOL:
                    nc.gpsimd.partition_broadcast(out_ap, in_ap, channels=channels)
                else:
                    scr = dscr.tile([1, 512], fp32, name=name)
                    nc.sync.dma_start(out=scr[:, :], in_=in_ap)
                    nc.sync.dma_start(out=out_ap,
                                      in_=scr[0:1, :].partition_broadcast(channels))

            def pool_or_dve(op, *args, **kw):
                getattr(nc.gpsimd if USE_POOL_OPS else nc.vector, op)(*args, **kw)

            # work: fp32 [128, 8, 1024]; x^T(half) -> t1 -> a -> t2 -> c -> t3 -> y
            work = p1.tile([P, HT, QL], fp32, tag="work")

            # DRAM scratch for K^T and V_aug
            kT_d = dpool.tile([H, S], f8, name="kT_d")
            q_d = dpool.tile([H, QL], f8, name="q_d")
            vaug = dpool.tile([S, NH, 66], bf16, name="vaug_d")

            def load_w(handle, eng=None, n_m=HT, dt=f8):
                # split per m-block so the first matmuls start after 1/8 of the DMA
                eng = eng or nc.scalar
                wt = p2.tile([P, HT, n_m * P], dt, tag="w", bufs=2, name="wt")
                for m in range(n_m):
                    eng.dma_start(out=wt[:, :, m * P:(m + 1) * P],
                                  in_=handle[:, m * P:(m + 1) * P]
                                  .rearrange("(t p) m -> p t m", p=P))
                return wt

            def mm_dr(ps, wt, m, rhs, qs, n_k=HT):
                """n_k/2 DoubleRow fp8 matmuls accumulating a full contraction."""
                mms = []
                for kp_ in range(n_k // 2):
                    mms.append(nc.tensor.matmul(
                        ps[:], wt[:, 2 * kp_:2 * kp_ + 2, m * P:(m + 1) * P],
                        rhs[:, 2 * kp_:2 * kp_ + 2, qs],
                        start=(kp_ == 0), stop=(kp_ == n_k // 2 - 1),
                        perf_mode=DR))
                return mms

            def mm_bf(ps, wt, m, rhs, qs, n_k=HT):
                """n_k plain matmuls accumulating a full contraction."""
                mms = []
                for kt in range(n_k):
                    mms.append(nc.tensor.matmul(
                        ps[:], wt[:, kt, m * P:(m + 1) * P], rhs[:, kt, qs],
                        start=(kt == 0), stop=(kt == n_k - 1)))
                return mms

            # ---------- Phase 1: Q^T = x @ wq16 / 16 + bq ----------
            q_f8 = p1.tile([P, HT, QL], f8, tag="qT")
            xqs = []
            for qn in range(QN):
                xq = p2.tile([P, HT, 512], f8, tag="xs", bufs=2, name="xq")
                xqs.append(xq)
            w = p2.tile([P, HT, HT * P], f8, tag="w", bufs=2, name="wt")
            nc.scalar.dma_start(out=w[:, :, 0:P], in_=w_sq[:, 0:P]
                                .rearrange("(t p) m -> p t m", p=P))
            nc.sync.dma_start(out=xqs[0][:], in_=xT[:, 0:512]
                              .rearrange("(t p) q -> p t q", p=P))
            for m in range(1, HT):
                nc.scalar.dma_start(out=w[:, :, m * P:(m + 1) * P],
                                    in_=w_sq[:, m * P:(m + 1) * P]
                                    .rearrange("(t p) m -> p t m", p=P))
            nc.sync.dma_start(out=xqs[1][:], in_=xT[:, 512:1024]
                              .rearrange("(t p) q -> p t q", p=P))
            for qn in range(QN):
                xq = xqs[qn]
                for m in range(HT):
                    pool_ = psA if m % 2 else psB
                    ps = pool_.tile([P, 512], fp32, tag="sp2" if m % 2 else "mm",
                                    name="psq")
                    mm_dr(ps, w, m, xq, slice(0, 512))
                    nc.vector.tensor_scalar(
                        out=q_f8[:, m, qn * 512:(qn + 1) * 512], in0=ps[:],
                        scalar1=IWS, scalar2=col(QB + m), op0=MULT, op1=ADD)

            nc.sync.dma_start(out=q_d[:, :].rearrange("(t p) q -> p t q", p=P),
                              in_=q_f8[:])

            # ---------- Phase 2: K^T and V (merged; shared x tiles) ----------
            wk = load_w(w_sk)
            wv = load_w(w_sv)
            k_anchor = None
            for sn in range(S // 512):
                if sn < 2:
                    xk = xqs[sn]  # first half of rotated seq = Q input, reuse
                else:
                    xk = p2.tile([P, HT, 512], f8, tag="xs", bufs=2, name="xk")
                    nc.sync.dma_start(out=xk[:], in_=xT[:, sn * 512:(sn + 1) * 512]
                                      .rearrange("(t p) q -> p t q", p=P))
                for m in range(HT):
                    pool_ = psA if m % 2 else psB
                    ps = pool_.tile([P, 512], fp32, tag="sp2" if m % 2 else "mm",
                                    name="psk")
                    mms = mm_dr(ps, wk, m, xk, slice(0, 512))
                    if k_anchor is None:
                        k_anchor = mms[0]
                    kt_t = p2.tile([P, 512], f8, tag="ktmp", name="kt_t")
                    nc.vector.tensor_scalar(
                        out=kt_t[:], in0=ps[:],
                        scalar1=IWS, scalar2=col(KB + m), op0=MULT, op1=ADD)
                    nc.sync.dma_start(out=kT_d[m * P:(m + 1) * P, sn * 512:(sn + 1) * 512],
                                      in_=kt_t[:])
                for c in range(4):
                    tt = 4 * sn + c
                    vt = p2.tile([P, NH, 66], bf16, tag="vv", bufs=2, name="vt")
                    for ds in range(2):
                        pool_ = psA if ds else psB
                        ps = pool_.tile([P, 512], fp32, tag="sp2" if ds else "mm",
                                        name="psv")
                        for kp_ in range(HT // 2):
                            nc.tensor.matmul(
                                ps[:], xk[:, 2 * kp_:2 * kp_ + 2, c * P:(c + 1) * P],
                                wv[:, 2 * kp_:2 * kp_ + 2, ds * 512:(ds + 1) * 512],
                                start=(kp_ == 0), stop=(kp_ == HT // 2 - 1),
                                perf_mode=DR)
                        nc.vector.tensor_scalar(
                            out=vt[:, ds * 8:(ds + 1) * 8, 0:64],
                            in0=ps[:].rearrange("p (h c) -> p h c", c=64),
                            scalar1=IWS, scalar2=None, op0=MULT)
                    nc.vector.memset(vt[:, :, 64:66], 1.0)
                    (nc.gpsimd if USE_POOL else nc.sync).dma_start(out=vaug[tt * P:(tt + 1) * P, :, :], in_=vt[:])

            # band tiles load on the gpsimd queue, held back behind K-proj
            # startup so they can't crowd the startup DMAs
            band_sb = None
            if band_kq:
                band_sb = p1.tile([P, len(band_kq), 512], f8, tag="band", name="band_sb")
                band_dma = nc.gpsimd.dma_start(out=band_sb[:],
                                               in_=band_d[:, :, :].rearrange("t p c -> p t c"))
                add_dep_helper(band_dma.ins, k_anchor.ins, sync=True,
                               reason="delay band load past startup")
            band_idx = {kq: i for i, kq in enumerate(band_kq)}

            # x residual halves load mid-attention (anchored below), by qn
            work_dmas = [
                nc.gpsimd.dma_start(out=work[:, :, 0:512],
                                    in_=xh[:, 0:512].rearrange("(t p) q -> p t q", p=P)),
                nc.gpsimd.dma_start(out=work[:, :, 512:1024],
                                    in_=xh[:, 512:1024].rearrange("(t p) q -> p t q", p=P))]

            ctx_sb = p1.tile([P, HT, QL], f8, tag="ctx")
            att_anchors = {}

            def attention(qn, hooks=None):
                hooks = hooks or {}
                for a in range(NH // 2):
                    # K and Q in dual-row d-split layout: head h of the pair on
                    # partitions h*32..h*32+31, head-dim split across 2 planes
                    kp = p2.tile([64, 2, S], f8, tag="kp", bufs=2, name="kp")
                    q2 = p2.tile([64, 2, 512], f8, tag="q2", bufs=2, name="q2")
                    for h in range(2):
                        nc.sync.dma_start(
                            out=q2[h * 32:(h + 1) * 32, :, :],
                            in_=q_d[a * P + h * 64:a * P + (h + 1) * 64,
                                    qn * 512:(qn + 1) * 512]
                            .rearrange("(two p) q -> p two q", two=2, p=32))
                    vp = p2.tile([P, ST // 2, 2, 2, 66], bf16, tag="vp", bufs=2,
                                 name="vp")
                    for c in range(4):
                        for h in range(2):
                            nc.sync.dma_start(
                                out=kp[h * 32:(h + 1) * 32, :, c * 512:(c + 1) * 512],
                                in_=kT_d[a * P + h * 64:a * P + (h + 1) * 64,
                                         c * 512:(c + 1) * 512]
                                .rearrange("(two p) s -> p two s", two=2, p=32))
                        nc.sync.dma_start(
                            out=vp[:, c * 2:(c + 1) * 2, :, :, :],
                            in_=vaug[c * 512:(c + 1) * 512, 2 * a:2 * a + 2, :]
                            .rearrange("(kt two p) h c -> p kt two h c", p=P, two=2))
                    cps = [psC.tile([65, 512], fp32, tag="ctx", name=f"ctxps{i}")
                           for i in range(2)]
                    for ktp in range(ST // 2):
                        pt = p4.tile([P, 2, 2, 512], bf16, tag="ptile", bufs=2, name="pt")
                        for two in range(2):
                            kt = 2 * ktp + two
                            sp2 = psA.tile([P, 2, 512], fp32, tag="sp2", name="sp2")
                            for hh in range(2):
                                if qn == 0 and kt == 0 and hh == 0 and a in (1, 2):
                                    att_anchors[a] = None  # filled after matmul below
                                has_band = (kt, qn) in band_idx
                                mm = nc.tensor.matmul(
                                    sp2[:, hh, :],
                                    kp[hh * 32:(hh + 1) * 32, :, kt * P:(kt + 1) * P],
                                    q2[hh * 32:(hh + 1) * 32, :, :],
                                    start=True, stop=not has_band,
                                    perf_mode=DR)
                                if att_anchors.get(a, 1) is None:
                                    att_anchors[a] = mm
                                if has_band:
                                    nc.tensor.matmul(sp2[:, hh, :], identt[:],
                                                     band_sb[:, band_idx[(kt, qn)], :],
                                                     start=False, stop=True)
                            # exp over both heads at once: [128, 2x512]
                            nc.scalar.activation(out=pt[:, two, :, :], in_=sp2[:, :, :],
                                                 func=mybir.ActivationFunctionType.Exp,
                                                 bias=col(NEG2), scale=0.125)
                            for hh in range(2):
                                nc.tensor.matmul(cps[hh][:], vp[:, ktp, two, hh, 0:65],
                                                 pt[:, two, hh, :],
                                                 start=(kt == 0), stop=(kt == ST - 1))
                    for hh in range(2):
                        cp = cps[hh]
                        rec = p3.tile([1, 512], fp32, tag="rows", bufs=2, name="rec")
                        nc.vector.reciprocal(out=rec[:], in_=cp[64:65, :])
                        bc = p2.tile([P, 512], fp32, tag="bc", bufs=2, name="bca")
                        bcast(bc[0:64, :], rec[:], 64, "scr_a")
                        nc.vector.tensor_mul(
                            out=ctx_sb[hh * 64:(hh + 1) * 64, a, qn * 512:(qn + 1) * 512],
                            in0=cp[0:64, :], in1=bc[0:64, :])
                    if a in hooks:
                        hooks[a]()

            # ---------- residual-add + LayerNorm helpers (transposed) ----------
            def layer_norm(qn, gcol, bcol, out_t=None, sq_act=False):
                """work[:, :, qn] holds t (fp32). Normalize in place; optional
                low-precision copy out. sq_act: squares on ACT (ACT-idle phases)."""
                qs = slice(qn * 512, (qn + 1) * 512)
                mean_ps = psC.tile([1, 512], fp32, tag="ctx", name="mean_ps")
                sq_ps = psC.tile([1, 512], fp32, tag="ctx", name="sq_ps")
                for kt in range(HT):
                    lnt = p3.tile([P, 2, 512], bf16, tag="lnt", bufs=2, name="lnt")
                    nc.vector.tensor_copy(out=lnt[:, 0, :], in_=work[:, kt, qs])
                    if sq_act:
                        nc.scalar.activation(out=lnt[:, 1, :], in_=work[:, kt, qs],
                                             func=mybir.ActivationFunctionType.Square)
                    else:
                        pool_or_dve("tensor_mul", out=lnt[:, 1, :],
                                    in0=work[:, kt, qs], in1=work[:, kt, qs])
                    nc.tensor.matmul(mean_ps[:], onest[:], lnt[:, 0, :],
                                     start=(kt == 0), stop=(kt == HT - 1))
                    nc.tensor.matmul(sq_ps[:], onest[:], lnt[:, 1, :],
                                     start=(kt == 0), stop=(kt == HT - 1))
                negmean = p3.tile([1, 512], fp32, tag="rows", bufs=2, name="negmean")
                nc.vector.tensor_scalar(out=negmean[:], in0=mean_ps[:],
                                        scalar1=-1.0 / H, scalar2=None, op0=MULT)
                msq = p3.tile([1, 512], fp32, tag="rows", bufs=2, name="msq")
                nc.vector.tensor_scalar(out=msq[:], in0=sq_ps[:],
                                        scalar1=1.0 / H, scalar2=None, op0=MULT)
                nm_bc = p2.tile([P, 512], fp32, tag="bc", name="nm_bc")
                bcast(nm_bc[:], negmean[:], P, "scr_nm")
                # nm broadcast issued; square it in place, then var/std/inv
                nc.vector.tensor_mul(out=negmean[:], in0=negmean[:], in1=negmean[:])
                nc.vector.tensor_sub(out=msq[:], in0=msq[:], in1=negmean[:])
                nc.scalar.activation(out=msq[:], in_=msq[:],
                                     func=mybir.ActivationFunctionType.Sqrt,
                                     bias=eps_t[:])
                nc.vector.reciprocal(out=msq[:], in_=msq[:])
                iv_bc = p2.tile([P, 512], fp32, tag="bc", name="iv_bc")
                bcast(iv_bc[:], msq[:], P, "scr_iv")
                for j in range(HT):
                    pool_or_dve("tensor_add", out=work[:, j, qs],
                                in0=work[:, j, qs], in1=nm_bc[:])
                    nc.vector.tensor_mul(out=work[:, j, qs], in0=work[:, j, qs],
                                         in1=iv_bc[:])
                    pool_or_dve("tensor_scalar",
                                out=work[:, j, qs], in0=work[:, j, qs],
                                scalar1=col(gcol + j), scalar2=col(bcol + j),
                                op0=MULT, op1=ADD)
                    if out_t is not None:
                        nc.vector.tensor_copy(out=out_t[:, j, qs],
                                              in_=work[:, j, qs])

            def proj_add_residual(w, rhs, qn, bcol=None):
                """work <- (proj of rhs via w16)/16 [+ bias] + work, per m tile."""
                qs = slice(qn * 512, (qn + 1) * 512)
                for m in range(HT):
                    ps = psB.tile([P, 512], fp32, tag="mm", name="pso")
                    mm_dr(ps, w, m, rhs, qs)
                    nc.vector.scalar_tensor_tensor(
                        out=work[:, m, qs], in0=ps[:], scalar=IWS,
                        in1=work[:, m, qs], op0=MULT, op1=ADD)
                    if bcol is not None:
                        pool_or_dve("tensor_scalar",
                                    out=work[:, m, qs], in0=work[:, m, qs],
                                    scalar1=col(bcol + m), scalar2=None, op0=ADD)

            # ---------- cross-attention building blocks ----------
            kc = p1.tile([P, HT, T], bf16, tag="kc")

            def kc_proj(wck):
                for m in range(HT):
                    ps = psB.tile([P, T], fp32, tag="mm", name="pskc")
                    for kt in range(HT):
                        nc.tensor.matmul(ps[:], wck[:, kt, m * P:(m + 1) * P],
                                         tg[:, kt, 0:T],
                                         start=(kt == 0), stop=(kt == HT - 1))
                    nc.vector.tensor_scalar(
                        out=kc[:, m, :], in0=ps[:],
                        scalar1=IWS * IWS, scalar2=col(CKB + m), op0=MULT, op1=ADD)

            vca = p2.tile([P, NH, 66], bf16, tag="vv", bufs=2, name="vca")

            def vca_proj(wcv):
                for ds in range(2):
                    ps = psB.tile([T, 512], fp32, tag="mm", name="psvc")
                    for kt in range(HT):
                        nc.tensor.matmul(ps[:], tg[:, kt, 0:T],
                                         wcv[:, kt, ds * 512:(ds + 1) * 512],
                                         start=(kt == 0), stop=(kt == HT - 1))
                    nc.vector.tensor_scalar(
                        out=vca[0:T, ds * 8:(ds + 1) * 8, 0:64],
                        in0=ps[:].rearrange("p (h c) -> p h c", c=64),
                        scalar1=IWS * IWS, scalar2=None, op0=MULT)
                nc.vector.memset(vca[0:T, :, 64:66], 1.0)

            ctxc = p1.tile([P, HT, QL], f8, tag="ctxc")
            a_f8 = p1.tile([P, HT, QL], f8, tag="a_f8")
            c_bf = p1.tile([P, HT, QL], bf16, tag="c_bf")

            def cross_block(qn, wcq, a, act_evac=False):
                qs = slice(qn * 512, (qn + 1) * 512)
                ps = psB.tile([P, 512], fp32, tag="mm", name="psqc")
                mm_dr(ps, wcq, a, a_f8, qs)
                qc_t = p2.tile([P, 512], bf16, tag="ktmp", name="qc_t")
                if act_evac:
                    nc.scalar.activation(out=qc_t[:], in_=ps[:],
                                         func=mybir.ActivationFunctionType.Identity,
                                         bias=col(CQB + a), scale=IWS)
                else:
                    nc.vector.tensor_scalar(
                        out=qc_t[:], in0=ps[:],
                        scalar1=IWS, scalar2=col(CQB + a), op0=MULT, op1=ADD)
                for hh in range(2):
                    sp = psA.tile([T, 512], fp32, tag="sp2", name="spc")
                    nc.tensor.matmul(sp[:], kc[hh * 64:(hh + 1) * 64, a, 0:T],
                                     qc_t[hh * 64:(hh + 1) * 64, :],
                                     start=True, stop=True)
                    pt = p4.tile([T, 512], bf16, tag="ptile", bufs=2, name="ptc")
                    nc.scalar.activation(out=pt[:], in_=sp[:],
                                         func=mybir.ActivationFunctionType.Exp,
                                         bias=col(NEG2, n=T), scale=0.125)
                    cp = psC.tile([65, 512], fp32, tag="ctx", name="cpc")
                    nc.tensor.matmul(cp[:], vca[0:T, 2 * a + hh, 0:65], pt[:],
                                     start=True, stop=True)
                    rec = p3.tile([1, 512], fp32, tag="rows", bufs=2, name="recc")
                    nc.vector.reciprocal(out=rec[:], in_=cp[64:65, :])
                    bc = p2.tile([P, 512], fp32, tag="bc", bufs=2, name="bcc")
                    bcast(bc[0:64, :], rec[:], 64, "scr_c")
                    nc.vector.tensor_mul(
                        out=ctxc[hh * 64:(hh + 1) * 64, a, qs],
                        in0=cp[0:64, :], in1=bc[0:64, :])

            # ---------- schedule ----------
            # attention qn0, with cross-K/V projections interleaved once their
            # weights (issued on the gpsimd queue at hook time) have arrived
            wck_box, wcv_box, wso_box = [], [], []
            attention(0, hooks={
                0: lambda: wso_box.append(load_w(w_so, eng=nc.gpsimd if USE_POOL else nc.sync)),
                3: lambda: wck_box.append(load_w(w_ck, eng=nc.gpsimd if USE_POOL else nc.sync)),
                4: lambda: kc_proj(wck_box[0]),
                5: lambda: wcv_box.append(load_w(w_cv, eng=nc.gpsimd if USE_POOL else nc.sync)),
                6: lambda: vca_proj(wcv_box[0]),
            })
            # qn0 out-proj + LN1 overlap qn1's (ACT-bound) attention
            proj_add_residual(wso_box[0], ctx_sb, 0)
            layer_norm(0, SLG, SLB, out_t=a_f8)
            wcq_box, wso2_box = [], []

            def att1_hook(a):
                if a == 0:
                    wcq_box.append(load_w(w_cq, eng=nc.gpsimd if USE_POOL else nc.sync))
                else:
                    cross_block(0, wcq_box[0], a - 1)
                if a == 4:
                    wso2_box.append(load_w(w_so, eng=nc.gpsimd if USE_POOL else nc.sync))

            attention(1, hooks={a: (lambda a=a: att1_hook(a)) for a in range(8)})
            cross_block(0, wcq_box[0], 7)
            proj_add_residual(wso2_box[0], ctx_sb, 1)
            layer_norm(1, SLG, SLB, out_t=a_f8)
            for a in range(HT):
                cross_block(1, wcq_box[0], a, act_evac=True)
            wco = load_w(w_co)
            proj_add_residual(wco, ctxc, 0, bcol=COB)
            layer_norm(0, CLG, CLB, out_t=c_bf, sq_act=True)
            proj_add_residual(wco, ctxc, 1, bcol=COB)
            layer_norm(1, CLG, CLB, out_t=c_bf, sq_act=True)

            # ---------- FFN, bf16 (chunk-outer: each weight block loads once) ----------
            for ch in range(FT // FC):
                inters = [p2.tile([P, FC, 512], bf16, tag="xs", bufs=2,
                                  name=f"inter{i}") for i in range(QN)]
                for mi in range(FC):
                    m = ch * FC + mi
                    wi = p3.tile([P, HT, P], bf16, tag="wi", bufs=2, name="wi")
                    nc.sync.dma_start(out=wi[:], in_=w_i[:, m * P:(m + 1) * P]
                                      .rearrange("(t p) c -> p t c", p=P))
                    for qn in range(QN):
                        qs = slice(qn * 512, (qn + 1) * 512)
                        pool_ = psA if qn else psB
                        ps = pool_.tile([P, 512], fp32, tag="sp2" if qn else "mm",
                                        name="psi")
                        mm_bf(ps, wi, 0, c_bf, qs)
                        nc.scalar.activation(out=inters[qn][:, mi, :], in_=ps[:],
                                             func=mybir.ActivationFunctionType.Gelu,
                                             bias=col(IB + m), scale=1.0)
                for mo in range(HT):
                    wo = p2.tile([P, FC, P], bf16, tag="wo", name="wo")
                    nc.sync.dma_start(out=wo[:], in_=w_o[ch * FC * P:(ch + 1) * FC * P,
                                                        mo * P:(mo + 1) * P]
                                      .rearrange("(t p) c -> p t c", p=P))
                    for qn in range(QN):
                        qs = slice(qn * 512, (qn + 1) * 512)
                        pool_ = psA if qn else psB
                        ps = pool_.tile([P, 512], fp32, tag="sp2" if qn else "mm",
                                        name="pso2")
                        mm_bf(ps, wo, 0, inters[qn], slice(0, 512), n_k=FC)
                        nc.vector.scalar_tensor_tensor(
                            out=work[:, mo, qs], in0=ps[:], scalar=1.0,
                            in1=work[:, mo, qs], op0=MULT, op1=ADD)
                        if ch == FT // FC - 1:
                            pool_or_dve("tensor_scalar",
                                        out=work[:, mo, qs], in0=work[:, mo, qs],
                                        scalar1=col(OB + mo), scalar2=None, op0=ADD)

            for qn in range(QN):
                layer_norm(qn, OLG, OLB, sq_act=True)
                qs = slice(qn * 512, (qn + 1) * 512)
                for j in range(HT):
                    nc.sync.dma_start(out=yT[j * P:(j + 1) * P, qs],
                                      in_=work[:, j, qs])

            for i, (a, mm) in enumerate(sorted(att_anchors.items())):
                add_dep_helper(work_dmas[i].ins, mm.ins, sync=True,
                               reason="residual load rides mid-attention")

    nc.compile()
    return nc, band_kq


def _get_program(er):
    key = int(er)
    if key not in _CACHE:
        _CACHE[key] = _build(key)
    return _CACHE[key]


def build_in_maps(inp, band_kq, er):
    x = inp["x"].astype(np.float32)
    B, S_, H_ = x.shape

    # host-side staging: H x H projections x16 in fp8 (dynamic range), the
    # /16 applied on-device at PSUM evacuation; FFN weights bf16 (fp8 noise
    # on the last block lands unattenuated on the output)
    wcast = {n: (inp[n].astype(np.float32) * WS).astype(F8)
             for n in ("sq_w", "sk_w", "sv_w", "so_w", "cq_w", "ck_w", "cv_w", "co_w")}
    wcast.update({n: inp[n].astype(np.float32).astype(BF) for n in ("i_w", "o_w")})
    so_b_eff = inp["so_b"].astype(np.float32) + inp["sv_b"].astype(np.float32) @ inp["so_w"].astype(np.float32)
    co_b_eff = inp["co_b"].astype(np.float32) + inp["cv_b"].astype(np.float32) @ inp["co_w"].astype(np.float32)
    pvec = np.zeros((P, PCOLS), np.float32)

    def pack(colbase, vec):
        v = np.asarray(vec, np.float32).reshape(-1, P)  # [k, 128]
        pvec[:, colbase:colbase + v.shape[0]] = v.T

    pack(QB, inp["sq_b"])
    pack(KB, inp["sk_b"])
    pack(SLG, inp["sln_g"]); pack(SLB, inp["sln_b"])
    pack(CQB, inp["cq_b"])
    pack(CKB, inp["ck_b"])
    pack(COB, co_b_eff)
    pack(CLG, inp["cln_g"]); pack(CLB, inp["cln_b"])
    pack(IB, inp["i_b"])
    pack(OB, inp["o_b"])
    pack(OLG, inp["oln_g"]); pack(OLB, inp["oln_b"])
    pvec[:, NEG2] = -2.0

    tags = inp["emb_table"].astype(np.float32)[np.asarray(inp["ent_ids"]).astype(np.int64)]  # [T, H]
    assert tags.shape[0] == 50, f"program compiled for 50 tags, got {tags.shape[0]}"
    tagsT = np.zeros((H, 64), F8)
    tagsT[:, :tags.shape[0]] = (tags.T * WS).astype(F8)
    ident = np.eye(P, dtype=F8)
    ones = np.ones((P, 1), BF)

    # band tiles in local (rotated) coords, per half: for half=1 the rotated
    # tail rows (k_rot >= S - QL) correspond to true keys k_rot - S.
    # NOTE: the +1 mask rides through exp(s/8 - 2) as exp((s+8)/8 - 2), so
    # the staged band value must be 8 (the exp scale) per masked element.
    nb = max(len(band_kq), 1)
    band_h = np.zeros((2, nb, P, 512), F8)
    if band_kq and er > 0:
        for i, (kt, qn) in enumerate(band_kq):
            k_rot = kt * P + np.arange(P)[:, None]
            q_rot = qn * 512 + np.arange(512)[None, :]
            d = k_rot - q_rot
            m0 = np.abs(d) <= er
            m1 = np.where(k_rot >= S_ - QL, np.abs(d - S_) <= er, m0)
            band_h[0, i] = (m0 * 8.0).astype(F8)
            band_h[1, i] = (m1 * 8.0).astype(F8)

    in_maps = []
    for c in range(8):
        b, half = divmod(c, 2)
        xt = x[b].T  # [H, S]
        rot = np.concatenate([xt[:, half * QL:], xt[:, :half * QL]], axis=1)
        in_maps.append({
            "xT": np.ascontiguousarray(rot).astype(F8),
            "xh": np.ascontiguousarray(rot[:, :QL]) + so_b_eff[:, None],
            "w_sq": wcast["sq_w"], "w_sk": wcast["sk_w"], "w_sv": wcast["sv_w"],
            "w_so": wcast["so_w"], "w_cq": wcast["cq_w"], "w_ck": wcast["ck_w"],
            "w_cv": wcast["cv_w"], "w_co": wcast["co_w"],
            "w_i": wcast["i_w"], "w_o": wcast["o_w"],
            "pvec": pvec, "tagsT": tagsT, "ident": ident, "ones": ones,
            "band": np.ascontiguousarray(band_h[half]),
        })
    return in_maps


def kernel(**inputs):
    inp = {k: np.asarray(v) for k, v in inputs.items()}
    x = inp["x"]
    B, S_, H_ = x.shape
    er = int(inp["ent_range"])
    nc, band_kq = _get_program(er)
    in_maps = build_in_maps(inp, band_kq, er)

    res = run_bass_kernel_spmd(nc, in_maps, core_ids=list(range(8)))
    out = np.empty((B, S_, H_), np.float32)
    for c in range(8):
        b, half = divmod(c, 2)
        out[b, half * QL:(half + 1) * QL, :] = res.results[c]["yT"].T
    return out


# revision 31
# speedup vs baseline: 1.0317x; 1.0317x over previous
"""Self-contained Trainium2 Bass kernel for nn_EntAttentionLayer.

Sharding: 8 cores = (batch 4) x (sequence half 2), no collectives.
Each core computes its [1024 tokens, 1024 hid] slice of the output
end-to-end: self-attention (banded mask) -> cross-attention to tag
embeddings -> FFN, each with residual + LayerNorm.

Device layout: activations kept transposed [hid(part), tok(free)].
  - the six H x H projections run in fp8e4m3 DoubleRow perf mode
    (2 contraction tiles per instruction); weights staged x16 on host
    (fp8 dynamic range), the 1/16 folded into PSUM evacuations. The
    FFN stays bf16: fp8 noise there lands directly on the output
    (no downstream LN to attenuate it) and blows the error budget.
  - scores computed transposed S^T[k, q] = (K^T slice).T @ (Q^T slice)
    in bf16; softmax exp(s/8 - 2) on ACT, batched over both heads of a
    pair ([128, 2x512] PSUM tile); the -2 shift cancels in the
    normalize and keeps p in range
  - band mask added via identity-matmul accumulation into PSUM (value
    8.0 pre-compensates the 1/8 exp scale)
  - sum_k exp folded into PV matmul via a ones-column appended to V
  - LayerNorm stats via bf16 ones-vector matmuls on [x | x^2] copies
  - [1,N] -> [128,N] broadcasts via gpsimd partition_broadcast (no DMA)
  - schedule: attention runs qn-outer; the qn0 out-proj + LN1 and the
    qn0 cross-attention blocks interleave into qn1's (ACT/exp-bound)
    attention window via per-head hooks
Per-core inputs are staged with the sequence ROTATED by half*1024 so all
8 cores run the identical program (band tiles are core-local data).
Residual/LN math in fp32.
"""

import sys

for _p in ("/opt/trn_rl_repo",):
    if _p not in sys.path:
        sys.path.insert(0, _p)

import numpy as np
import ml_dtypes

import concourse.bacc as bacc
import concourse.mybir as mybir
import concourse.tile as tile
from concourse.tile import add_dep_helper
from concourse.bass_utils import run_bass_kernel_spmd

BF = ml_dtypes.bfloat16
F8 = ml_dtypes.float8_e4m3
fp32 = mybir.dt.float32
bf16 = mybir.dt.bfloat16
f8 = mybir.dt.float8e4
DR = mybir.MatmulPerfMode.DoubleRow
ADD = mybir.AluOpType.add
MULT = mybir.AluOpType.mult

H = 1024          # hidden
S = 2048          # full sequence
QL = 1024         # per-core query tokens
FFN = 4096
NH, HD = 16, 64
P = 128
HT = H // P       # 8 hid tiles
ST = S // P       # 16 seq tiles
QN = QL // 512    # 2 q blocks of 512
EPS = 1e-12
WS = 16.0         # fp8 weight staging scale
IWS = 1.0 / WS

# pvec column offsets (per-partition param pack, [128, PCOLS] fp32)
QB, KB, SLG, SLB = 0, 8, 16, 24
CQB, CKB, COB, CLG, CLB = 32, 40, 48, 56, 64
IB, OB, OLG, OLB = 72, 104, 112, 120
NEG2 = 128  # column of -2.0 (exp shift)
PCOLS = 129

USE_POOL = False   # gpsimd partition_broadcast + swdge DMA
USE_POOL_OPS = False  # gpsimd elementwise ops (slow on HW: software Q7 path)
_CACHE = {}


def _band_needed(er):
    """(kt, qn) pairs, in local (rotated) coords, where the band tile can be
    nonzero for either half. Core-independent."""
    out = []
    for kt in range(ST):
        for qn in range(QN):
            lo, hi = qn * 512 - er, qn * 512 + 511 + er
            k0, k1 = kt * P, kt * P + 127
            if (k0 <= hi and k1 >= lo) or (k0 - S <= hi and k1 - S >= lo):
                out.append((kt, qn))
    return out


def _build(er):
    """Build + bacc-compile the per-core program. er = ent_range (>=0)."""
    band_kq = _band_needed(er) if er > 0 else []
    nc = bacc.Bacc()

    # ---- DRAM I/O ----
    xT = nc.dram_tensor("xT", [H, S], f8, kind="ExternalInput")
    xh = nc.dram_tensor("xh", [H, QL], fp32, kind="ExternalInput")
    w_sq = nc.dram_tensor("w_sq", [H, H], f8, kind="ExternalInput")
    w_sk = nc.dram_tensor("w_sk", [H, H], f8, kind="ExternalInput")
    w_sv = nc.dram_tensor("w_sv", [H, H], f8, kind="ExternalInput")
    w_so = nc.dram_tensor("w_so", [H, H], f8, kind="ExternalInput")
    w_cq = nc.dram_tensor("w_cq", [H, H], f8, kind="ExternalInput")
    w_ck = nc.dram_tensor("w_ck", [H, H], f8, kind="ExternalInput")
    w_cv = nc.dram_tensor("w_cv", [H, H], f8, kind="ExternalInput")
    w_co = nc.dram_tensor("w_co", [H, H], f8, kind="ExternalInput")
    w_i = nc.dram_tensor("w_i", [H, FFN], bf16, kind="ExternalInput")
    w_o = nc.dram_tensor("w_o", [FFN, H], bf16, kind="ExternalInput")
    pvec = nc.dram_tensor("pvec", [P, PCOLS], fp32, kind="ExternalInput")
    tagsT = nc.dram_tensor("tagsT", [H, 64], f8, kind="ExternalInput")
    ident_d = nc.dram_tensor("ident", [P, P], f8, kind="ExternalInput")
    ones_d = nc.dram_tensor("ones", [P, 1], bf16, kind="ExternalInput")
    nb = max(len(band_kq), 1)
    band_d = nc.dram_tensor("band", [nb, P, 512], f8, kind="ExternalInput")
    yT = nc.dram_tensor("yT", [H, QL], fp32, kind="ExternalOutput")

    T = 50  # tags count (tagsT padded to 64 cols)
    FT = FFN // P  # 32
    FC = 8         # FFN m-tiles per chunk (4 chunks)

    with tile.TileContext(nc) as tc:
        with tc.tile_pool(name="p1", bufs=1) as p1, \
             tc.tile_pool(name="p2", bufs=2) as p2, \
             tc.tile_pool(name="p3", bufs=3) as p3, \
             tc.tile_pool(name="p4", bufs=4) as p4, \
             tc.tile_pool(name="psA", bufs=2, space="PSUM") as psA, \
             tc.tile_pool(name="psB", bufs=2, space="PSUM") as psB, \
             tc.tile_pool(name="psC", bufs=2, space="PSUM") as psC, \
             tc.tile_pool(name="dram", bufs=1, space="DRAM") as dpool, \
             tc.tile_pool(name="dscr", bufs=4, space="DRAM") as dscr:

            # ---- constants ----
            identt = p1.tile([P, P], f8, tag="ident")
            nc.sync.dma_start(out=identt[:], in_=ident_d[:, :])
            onest = p1.tile([P, 1], bf16, tag="ones")
            nc.sync.dma_start(out=onest[:], in_=ones_d[:, :])
            pv = p1.tile([P, PCOLS], fp32, tag="pvec")
            nc.sync.dma_start(out=pv[:], in_=pvec[:, :])
            tg = p1.tile([P, HT, 64], f8, tag="tags")
            nc.sync.dma_start(out=tg[:], in_=tagsT[:, :].rearrange("(t p) c -> p t c", p=P))
            eps_t = p1.tile([1, 1], fp32, tag="eps")
            nc.vector.memset(eps_t[:], EPS)

            def col(c, n=P):
                return pv[0:n, c:c + 1]

            def bcast(out_ap, in_ap, channels, name):
                if USE_POOL:
                    nc.gpsimd.partition_broadcast(out_ap, in_ap, channels=channels)
                else:
                    scr = dscr.tile([1, 512], fp32, name=name)
                    nc.sync.dma_start(out=scr[:, :], in_=in_ap)
                    nc.sync.dma_start(out=out_ap,
                                      in_=scr[0:1, :].partition_broadcast(channels))

            def pool_or_dve(op, *args, **kw):
                getattr(nc.gpsimd if USE_POOL_OPS else nc.vector, op)(*args, **kw)

            # work: fp32 [128, 8, 1024]; x^T(half) -> t1 -> a -> t2 -> c -> t3 -> y
            work = p1.tile([P, HT, QL], fp32, tag="work")

            # DRAM scratch for K^T and V_aug
            kT_d = dpool.tile([H, S], f8, name="kT_d")
            q_d = dpool.tile([H, QL], f8, name="q_d")
            vaug = dpool.tile([S, NH, 66], bf16, name="vaug_d")

            def load_w(handle, eng=None, n_m=HT, dt=f8):
                # split per m-block so the first matmuls start after 1/8 of the DMA
                eng = eng or nc.scalar
                wt = p2.tile([P, HT, n_m * P], dt, tag="w", bufs=2, name="wt")
                for m in range(n_m):
                    eng.dma_start(out=wt[:, :, m * P:(m + 1) * P],
                                  in_=handle[:, m * P:(m + 1) * P]
                                  .rearrange("(t p) m -> p t m", p=P))
                return wt

            def mm_dr(ps, wt, m, rhs, qs, n_k=HT):
                """n_k/2 DoubleRow fp8 matmuls accumulating a full contraction."""
                mms = []
                for kp_ in range(n_k // 2):
                    mms.append(nc.tensor.matmul(
                        ps[:], wt[:, 2 * kp_:2 * kp_ + 2, m * P:(m + 1) * P],
                        rhs[:, 2 * kp_:2 * kp_ + 2, qs],
                        start=(kp_ == 0), stop=(kp_ == n_k // 2 - 1),
                        perf_mode=DR))
                return mms

            def mm_bf(ps, wt, m, rhs, qs, n_k=HT):
                """n_k plain matmuls accumulating a full contraction."""
                mms = []
                for kt in range(n_k):
                    mms.append(nc.tensor.matmul(
                        ps[:], wt[:, kt, m * P:(m + 1) * P], rhs[:, kt, qs],
                        start=(kt == 0), stop=(kt == n_k - 1)))
                return mms

            # ---------- Phase 1: Q^T = x @ wq16 / 16 + bq ----------
            q_f8 = p1.tile([P, HT, QL], f8, tag="qT")
            xqs = []
            for qn in range(QN):
                xq = p2.tile([P, HT, 512], f8, tag="xs", bufs=2, name="xq")
                xqs.append(xq)
            w = p2.tile([P, HT, HT * P], f8, tag="w", bufs=2, name="wt")
            nc.scalar.dma_start(out=w[:, :, 0:P], in_=w_sq[:, 0:P]
                                .rearrange("(t p) m -> p t m", p=P))
            nc.sync.dma_start(out=xqs[0][:], in_=xT[:, 0:512]
                              .rearrange("(t p) q -> p t q", p=P))
            for m in range(1, HT):
                nc.scalar.dma_start(out=w[:, :, m * P:(m + 1) * P],
                                    in_=w_sq[:, m * P:(m + 1) * P]
                                    .rearrange("(t p) m -> p t m", p=P))
            nc.sync.dma_start(out=xqs[1][:], in_=xT[:, 512:1024]
                              .rearrange("(t p) q -> p t q", p=P))
            for qn in range(QN):
                xq = xqs[qn]
                for m in range(HT):
                    pool_ = psA if m % 2 else psB
                    ps = pool_.tile([P, 512], fp32, tag="sp2" if m % 2 else "mm",
                                    name="psq")
                    mm_dr(ps, w, m, xq, slice(0, 512))
                    nc.vector.tensor_scalar(
                        out=q_f8[:, m, qn * 512:(qn + 1) * 512], in0=ps[:],
                        scalar1=IWS, scalar2=col(QB + m), op0=MULT, op1=ADD)

            nc.sync.dma_start(out=q_d[:, :].rearrange("(t p) q -> p t q", p=P),
                              in_=q_f8[:])

            # ---------- Phase 2: K^T and V (merged; shared x tiles) ----------
            wk = load_w(w_sk)
            wv = load_w(w_sv)
            k_anchor = None
            for sn in range(S // 512):
                if sn < 2:
                    xk = xqs[sn]  # first half of rotated seq = Q input, reuse
                else:
                    xk = p2.tile([P, HT, 512], f8, tag="xs", bufs=2, name="xk")
                    nc.sync.dma_start(out=xk[:], in_=xT[:, sn * 512:(sn + 1) * 512]
                                      .rearrange("(t p) q -> p t q", p=P))
                for m in range(HT):
                    pool_ = psA if m % 2 else psB
                    ps = pool_.tile([P, 512], fp32, tag="sp2" if m % 2 else "mm",
                                    name="psk")
                    mms = mm_dr(ps, wk, m, xk, slice(0, 512))
                    if k_anchor is None:
                        k_anchor = mms[0]
                    kt_t = p2.tile([P, 512], f8, tag="ktmp", name="kt_t")
                    nc.vector.tensor_scalar(
                        out=kt_t[:], in0=ps[:],
                        scalar1=IWS, scalar2=col(KB + m), op0=MULT, op1=ADD)
                    nc.sync.dma_start(out=kT_d[m * P:(m + 1) * P, sn * 512:(sn + 1) * 512],
                                      in_=kt_t[:])
                for c in range(4):
                    tt = 4 * sn + c
                    vt = p2.tile([P, NH, 66], bf16, tag="vv", bufs=2, name="vt")
                    for ds in range(2):
                        pool_ = psA if ds else psB
                        ps = pool_.tile([P, 512], fp32, tag="sp2" if ds else "mm",
                                        name="psv")
                        for kp_ in range(HT // 2):
                            nc.tensor.matmul(
                                ps[:], xk[:, 2 * kp_:2 * kp_ + 2, c * P:(c + 1) * P],
                                wv[:, 2 * kp_:2 * kp_ + 2, ds * 512:(ds + 1) * 512],
                                start=(kp_ == 0), stop=(kp_ == HT // 2 - 1),
                                perf_mode=DR)
                        nc.vector.tensor_scalar(
                            out=vt[:, ds * 8:(ds + 1) * 8, 0:64],
                            in0=ps[:].rearrange("p (h c) -> p h c", c=64),
                            scalar1=IWS, scalar2=None, op0=MULT)
                    nc.vector.memset(vt[:, :, 64:66], 1.0)
                    (nc.gpsimd if USE_POOL else nc.sync).dma_start(out=vaug[tt * P:(tt + 1) * P, :, :], in_=vt[:])

            # band tiles load on the gpsimd queue, held back behind K-proj
            # startup so they can't crowd the startup DMAs
            band_sb = None
            if band_kq:
                band_sb = p1.tile([P, len(band_kq), 512], f8, tag="band", name="band_sb")
                band_dma = nc.gpsimd.dma_start(out=band_sb[:],
                                               in_=band_d[:, :, :].rearrange("t p c -> p t c"))
                add_dep_helper(band_dma.ins, k_anchor.ins, sync=True,
                               reason="delay band load past startup")
            band_idx = {kq: i for i, kq in enumerate(band_kq)}

            # x residual halves load mid-attention (anchored below), by qn
            work_dmas = [
                nc.gpsimd.dma_start(out=work[:, :, 0:512],
                                    in_=xh[:, 0:512].rearrange("(t p) q -> p t q", p=P)),
                nc.gpsimd.dma_start(out=work[:, :, 512:1024],
                                    in_=xh[:, 512:1024].rearrange("(t p) q -> p t q", p=P))]

            ctx_sb = p1.tile([P, HT, QL], f8, tag="ctx")
            att_anchors = {}

            def attention(qn, hooks=None):
                hooks = hooks or {}
                for a in range(NH // 2):
                    # K and Q in dual-row d-split layout: head h of the pair on
                    # partitions h*32..h*32+31, head-dim split across 2 planes
                    kp = p2.tile([64, 2, S], f8, tag="kp", bufs=2, name="kp")
                    q2 = p2.tile([64, 2, 512], f8, tag="q2", bufs=2, name="q2")
                    for h in range(2):
                        nc.sync.dma_start(
                            out=q2[h * 32:(h + 1) * 32, :, :],
                            in_=q_d[a * P + h * 64:a * P + (h + 1) * 64,
                                    qn * 512:(qn + 1) * 512]
                            .rearrange("(two p) q -> p two q", two=2, p=32))
                    vp = p2.tile([P, ST // 2, 2, 2, 66], bf16, tag="vp", bufs=2,
                                 name="vp")
                    # single whole-range loads: kT_d/vaug are fully written
                    # before attention starts, and fewer DMA issues decongest
                    # the SP sequencer (565ns each, serialized)
                    for h in range(2):
                        nc.sync.dma_start(
                            out=kp[h * 32:(h + 1) * 32, :, :],
                            in_=kT_d[a * P + h * 64:a * P + (h + 1) * 64, :]
                            .rearrange("(two p) s -> p two s", two=2, p=32))
                    nc.sync.dma_start(
                        out=vp[:],
                        in_=vaug[:, 2 * a:2 * a + 2, :]
                        .rearrange("(kt two p) h c -> p kt two h c", p=P, two=2))
                    cps = [psC.tile([65, 512], fp32, tag="ctx", name=f"ctxps{i}")
                           for i in range(2)]
                    for ktp in range(ST // 2):
                        pt = p4.tile([P, 2, 2, 512], bf16, tag="ptile", bufs=2, name="pt")
                        for two in range(2):
                            kt = 2 * ktp + two
                            sp2 = psA.tile([P, 2, 512], fp32, tag="sp2", name="sp2")
                            for hh in range(2):
                                if qn == 0 and kt == 0 and hh == 0 and a in (1, 2):
                                    att_anchors[a] = None  # filled after matmul below
                                has_band = (kt, qn) in band_idx
                                mm = nc.tensor.matmul(
                                    sp2[:, hh, :],
                                    kp[hh * 32:(hh + 1) * 32, :, kt * P:(kt + 1) * P],
                                    q2[hh * 32:(hh + 1) * 32, :, :],
                                    start=True, stop=not has_band,
                                    perf_mode=DR)
                                if att_anchors.get(a, 1) is None:
                                    att_anchors[a] = mm
                                if has_band:
                                    nc.tensor.matmul(sp2[:, hh, :], identt[:],
                                                     band_sb[:, band_idx[(kt, qn)], :],
                                                     start=False, stop=True)
                            # exp over both heads at once: [128, 2x512]
                            nc.scalar.activation(out=pt[:, two, :, :], in_=sp2[:, :, :],
                                                 func=mybir.ActivationFunctionType.Exp,
                                                 bias=col(NEG2), scale=0.125)
                            for hh in range(2):
                                nc.tensor.matmul(cps[hh][:], vp[:, ktp, two, hh, 0:65],
                                                 pt[:, two, hh, :],
                                                 start=(kt == 0), stop=(kt == ST - 1))
                    for hh in range(2):
                        cp = cps[hh]
                        rec = p3.tile([1, 512], fp32, tag="rows", bufs=2, name="rec")
                        nc.vector.reciprocal(out=rec[:], in_=cp[64:65, :])
                        bc = p2.tile([P, 512], fp32, tag="bc", bufs=2, name="bca")
                        bcast(bc[0:64, :], rec[:], 64, "scr_a")
                        nc.vector.tensor_mul(
                            out=ctx_sb[hh * 64:(hh + 1) * 64, a, qn * 512:(qn + 1) * 512],
                            in0=cp[0:64, :], in1=bc[0:64, :])
                    if a in hooks:
                        hooks[a]()

            # ---------- residual-add + LayerNorm helpers (transposed) ----------
            def layer_norm(qn, gcol, bcol, out_t=None, sq_act=False):
                """work[:, :, qn] holds t (fp32). Normalize in place; optional
                low-precision copy out. sq_act: squares on ACT (ACT-idle phases)."""
                qs = slice(qn * 512, (qn + 1) * 512)
                mean_ps = psC.tile([1, 512], fp32, tag="ctx", name="mean_ps")
                sq_ps = psC.tile([1, 512], fp32, tag="ctx", name="sq_ps")
                for kt in range(HT):
                    lnt = p3.tile([P, 2, 512], bf16, tag="lnt", bufs=2, name="lnt")
                    nc.vector.tensor_copy(out=lnt[:, 0, :], in_=work[:, kt, qs])
                    if sq_act:
                        nc.scalar.activation(out=lnt[:, 1, :], in_=work[:, kt, qs],
                                             func=mybir.ActivationFunctionType.Square)
                    else:
                        pool_or_dve("tensor_mul", out=lnt[:, 1, :],
                                    in0=work[:, kt, qs], in1=work[:, kt, qs])
                    nc.tensor.matmul(mean_ps[:], onest[:], lnt[:, 0, :],
                                     start=(kt == 0), stop=(kt == HT - 1))
                    nc.tensor.matmul(sq_ps[:], onest[:], lnt[:, 1, :],
                                     start=(kt == 0), stop=(kt == HT - 1))
                negmean = p3.tile([1, 512], fp32, tag="rows", bufs=2, name="negmean")
                nc.vector.tensor_scalar(out=negmean[:], in0=mean_ps[:],
                                        scalar1=-1.0 / H, scalar2=None, op0=MULT)
                msq = p3.tile([1, 512], fp32, tag="rows", bufs=2, name="msq")
                nc.vector.tensor_scalar(out=msq[:], in0=sq_ps[:],
                                        scalar1=1.0 / H, scalar2=None, op0=MULT)
                nm_bc = p2.tile([P, 512], fp32, tag="bc", name="nm_bc")
                bcast(nm_bc[:], negmean[:], P, "scr_nm")
                # nm broadcast issued; square it in place, then var/std/inv
                nc.vector.tensor_mul(out=negmean[:], in0=negmean[:], in1=negmean[:])
                nc.vector.tensor_sub(out=msq[:], in0=msq[:], in1=negmean[:])
                nc.scalar.activation(out=msq[:], in_=msq[:],
                                     func=mybir.ActivationFunctionType.Sqrt,
                                     bias=eps_t[:])
                nc.vector.reciprocal(out=msq[:], in_=msq[:])
                iv_bc = p2.tile([P, 512], fp32, tag="bc", name="iv_bc")
                bcast(iv_bc[:], msq[:], P, "scr_iv")
                for j in range(HT):
                    pool_or_dve("tensor_add", out=work[:, j, qs],
                                in0=work[:, j, qs], in1=nm_bc[:])
                    nc.vector.tensor_mul(out=work[:, j, qs], in0=work[:, j, qs],
                                         in1=iv_bc[:])
                    pool_or_dve("tensor_scalar",
                                out=work[:, j, qs], in0=work[:, j, qs],
                                scalar1=col(gcol + j), scalar2=col(bcol + j),
                                op0=MULT, op1=ADD)
                    if out_t is not None:
                        nc.vector.tensor_copy(out=out_t[:, j, qs],
                                              in_=work[:, j, qs])

            def proj_add_residual(w, rhs, qn, bcol=None):
                """work <- (proj of rhs via w16)/16 [+ bias] + work, per m tile."""
                qs = slice(qn * 512, (qn + 1) * 512)
                for m in range(HT):
                    ps = psB.tile([P, 512], fp32, tag="mm", name="pso")
                    mm_dr(ps, w, m, rhs, qs)
                    nc.vector.scalar_tensor_tensor(
                        out=work[:, m, qs], in0=ps[:], scalar=IWS,
                        in1=work[:, m, qs], op0=MULT, op1=ADD)
                    if bcol is not None:
                        pool_or_dve("tensor_scalar",
                                    out=work[:, m, qs], in0=work[:, m, qs],
                                    scalar1=col(bcol + m), scalar2=None, op0=ADD)

            # ---------- cross-attention building blocks ----------
            kc = p1.tile([P, HT, T], bf16, tag="kc")

            def kc_proj(wck):
                for m in range(HT):
                    ps = psB.tile([P, T], fp32, tag="mm", name="pskc")
                    for kt in range(HT):
                        nc.tensor.matmul(ps[:], wck[:, kt, m * P:(m + 1) * P],
                                         tg[:, kt, 0:T],
                                         start=(kt == 0), stop=(kt == HT - 1))
                    nc.vector.tensor_scalar(
                        out=kc[:, m, :], in0=ps[:],
                        scalar1=IWS * IWS, scalar2=col(CKB + m), op0=MULT, op1=ADD)

            vca = p2.tile([P, NH, 66], bf16, tag="vv", bufs=2, name="vca")

            def vca_proj(wcv):
                for ds in range(2):
                    ps = psB.tile([T, 512], fp32, tag="mm", name="psvc")
                    for kt in range(HT):
                        nc.tensor.matmul(ps[:], tg[:, kt, 0:T],
                                         wcv[:, kt, ds * 512:(ds + 1) * 512],
                                         start=(kt == 0), stop=(kt == HT - 1))
                    nc.vector.tensor_scalar(
                        out=vca[0:T, ds * 8:(ds + 1) * 8, 0:64],
                        in0=ps[:].rearrange("p (h c) -> p h c", c=64),
                        scalar1=IWS * IWS, scalar2=None, op0=MULT)
                nc.vector.memset(vca[0:T, :, 64:66], 1.0)

            ctxc = p1.tile([P, HT, QL], f8, tag="ctxc")
            a_f8 = p1.tile([P, HT, QL], f8, tag="a_f8")
            c_bf = p1.tile([P, HT, QL], bf16, tag="c_bf")

            def cross_block(qn, wcq, a, act_evac=False):
                qs = slice(qn * 512, (qn + 1) * 512)
                ps = psB.tile([P, 512], fp32, tag="mm", name="psqc")
                mm_dr(ps, wcq, a, a_f8, qs)
                qc_t = p2.tile([P, 512], bf16, tag="ktmp", name="qc_t")
                if act_evac:
                    nc.scalar.activation(out=qc_t[:], in_=ps[:],
                                         func=mybir.ActivationFunctionType.Identity,
                                         bias=col(CQB + a), scale=IWS)
                else:
                    nc.vector.tensor_scalar(
                        out=qc_t[:], in0=ps[:],
                        scalar1=IWS, scalar2=col(CQB + a), op0=MULT, op1=ADD)
                for hh in range(2):
                    sp = psA.tile([T, 512], fp32, tag="sp2", name="spc")
                    nc.tensor.matmul(sp[:], kc[hh * 64:(hh + 1) * 64, a, 0:T],
                                     qc_t[hh * 64:(hh + 1) * 64, :],
                                     start=True, stop=True)
                    pt = p4.tile([T, 512], bf16, tag="ptile", bufs=2, name="ptc")
                    nc.scalar.activation(out=pt[:], in_=sp[:],
                                         func=mybir.ActivationFunctionType.Exp,
                                         bias=col(NEG2, n=T), scale=0.125)
                    cp = psC.tile([65, 512], fp32, tag="ctx", name="cpc")
                    nc.tensor.matmul(cp[:], vca[0:T, 2 * a + hh, 0:65], pt[:],
                                     start=True, stop=True)
                    rec = p3.tile([1, 512], fp32, tag="rows", bufs=2, name="recc")
                    nc.vector.reciprocal(out=rec[:], in_=cp[64:65, :])
                    bc = p2.tile([P, 512], fp32, tag="bc", bufs=2, name="bcc")
                    bcast(bc[0:64, :], rec[:], 64, "scr_c")
                    nc.vector.tensor_mul(
                        out=ctxc[hh * 64:(hh + 1) * 64, a, qs],
                        in0=cp[0:64, :], in1=bc[0:64, :])

            # ---------- schedule ----------
            # attention qn0, with cross-K/V projections interleaved once their
            # weights (issued on the gpsimd queue at hook time) have arrived
            wck_box, wcv_box, wso_box = [], [], []
            attention(0, hooks={
                0: lambda: wso_box.append(load_w(w_so, eng=nc.gpsimd if USE_POOL else nc.sync)),
                3: lambda: wck_box.append(load_w(w_ck, eng=nc.gpsimd if USE_POOL else nc.sync)),
                4: lambda: kc_proj(wck_box[0]),
                5: lambda: wcv_box.append(load_w(w_cv, eng=nc.gpsimd if USE_POOL else nc.sync)),
                6: lambda: vca_proj(wcv_box[0]),
            })
            # qn0 out-proj + LN1 overlap qn1's (ACT-bound) attention
            proj_add_residual(wso_box[0], ctx_sb, 0)
            layer_norm(0, SLG, SLB, out_t=a_f8)
            wcq_box, wso2_box = [], []

            def att1_hook(a):
                if a == 0:
                    wcq_box.append(load_w(w_cq, eng=nc.gpsimd if USE_POOL else nc.sync))
                else:
                    cross_block(0, wcq_box[0], a - 1)
                if a == 4:
                    wso2_box.append(load_w(w_so, eng=nc.gpsimd if USE_POOL else nc.sync))

            attention(1, hooks={a: (lambda a=a: att1_hook(a)) for a in range(8)})
            cross_block(0, wcq_box[0], 7)
            proj_add_residual(wso2_box[0], ctx_sb, 1)
            layer_norm(1, SLG, SLB, out_t=a_f8)
            for a in range(HT):
                cross_block(1, wcq_box[0], a, act_evac=True)
            wco = load_w(w_co)
            proj_add_residual(wco, ctxc, 0, bcol=COB)
            layer_norm(0, CLG, CLB, out_t=c_bf, sq_act=True)
            proj_add_residual(wco, ctxc, 1, bcol=COB)
            layer_norm(1, CLG, CLB, out_t=c_bf, sq_act=True)

            # ---------- FFN, bf16 (chunk-outer: each weight block loads once) ----------
            for ch in range(FT // FC):
                inters = [p2.tile([P, FC, 512], bf16, tag="xs", bufs=2,
                                  name=f"inter{i}") for i in range(QN)]
                for mi in range(FC):
                    m = ch * FC + mi
                    wi = p3.tile([P, HT, P], bf16, tag="wi", bufs=2, name="wi")
                    nc.sync.dma_start(out=wi[:], in_=w_i[:, m * P:(m + 1) * P]
                                      .rearrange("(t p) c -> p t c", p=P))
                    for qn in range(QN):
                        qs = slice(qn * 512, (qn + 1) * 512)
                        pool_ = psA if qn else psB
                        ps = pool_.tile([P, 512], fp32, tag="sp2" if qn else "mm",
                                        name="psi")
                        mm_bf(ps, wi, 0, c_bf, qs)
                        nc.scalar.activation(out=inters[qn][:, mi, :], in_=ps[:],
                                             func=mybir.ActivationFunctionType.Gelu,
                                             bias=col(IB + m), scale=1.0)
                for mo in range(HT):
                    wo = p2.tile([P, FC, P], bf16, tag="wo", name="wo")
                    nc.sync.dma_start(out=wo[:], in_=w_o[ch * FC * P:(ch + 1) * FC * P,
                                                        mo * P:(mo + 1) * P]
                                      .rearrange("(t p) c -> p t c", p=P))
                    for qn in range(QN):
                        qs = slice(qn * 512, (qn + 1) * 512)
                        pool_ = psA if qn else psB
                        ps = pool_.tile([P, 512], fp32, tag="sp2" if qn else "mm",
                                        name="pso2")
                        mm_bf(ps, wo, 0, inters[qn], slice(0, 512), n_k=FC)
                        nc.vector.scalar_tensor_tensor(
                            out=work[:, mo, qs], in0=ps[:], scalar=1.0,
                            in1=work[:, mo, qs], op0=MULT, op1=ADD)
                        if ch == FT // FC - 1:
                            pool_or_dve("tensor_scalar",
                                        out=work[:, mo, qs], in0=work[:, mo, qs],
                                        scalar1=col(OB + mo), scalar2=None, op0=ADD)

            for qn in range(QN):
                layer_norm(qn, OLG, OLB, sq_act=True)
                qs = slice(qn * 512, (qn + 1) * 512)
                for j in range(HT):
                    nc.sync.dma_start(out=yT[j * P:(j + 1) * P, qs],
                                      in_=work[:, j, qs])

            for i, (a, mm) in enumerate(sorted(att_anchors.items())):
                add_dep_helper(work_dmas[i].ins, mm.ins, sync=True,
                               reason="residual load rides mid-attention")

    nc.compile()
    return nc, band_kq


def _get_program(er):
    key = int(er)
    if key not in _CACHE:
        _CACHE[key] = _build(key)
    return _CACHE[key]


def build_in_maps(inp, band_kq, er):
    x = inp["x"].astype(np.float32)
    B, S_, H_ = x.shape

    # host-side staging: H x H projections x16 in fp8 (dynamic range), the
    # /16 applied on-device at PSUM evacuation; FFN weights bf16 (fp8 noise
    # on the last block lands unattenuated on the output)
    wcast = {n: (inp[n].astype(np.float32) * WS).astype(F8)
             for n in ("sq_w", "sk_w", "sv_w", "so_w", "cq_w", "ck_w", "cv_w", "co_w")}
    wcast.update({n: inp[n].astype(np.float32).astype(BF) for n in ("i_w", "o_w")})
    so_b_eff = inp["so_b"].astype(np.float32) + inp["sv_b"].astype(np.float32) @ inp["so_w"].astype(np.float32)
    co_b_eff = inp["co_b"].astype(np.float32) + inp["cv_b"].astype(np.float32) @ inp["co_w"].astype(np.float32)
    pvec = np.zeros((P, PCOLS), np.float32)

    def pack(colbase, vec):
        v = np.asarray(vec, np.float32).reshape(-1, P)  # [k, 128]
        pvec[:, colbase:colbase + v.shape[0]] = v.T

    pack(QB, inp["sq_b"])
    pack(KB, inp["sk_b"])
    pack(SLG, inp["sln_g"]); pack(SLB, inp["sln_b"])
    pack(CQB, inp["cq_b"])
    pack(CKB, inp["ck_b"])
    pack(COB, co_b_eff)
    pack(CLG, inp["cln_g"]); pack(CLB, inp["cln_b"])
    pack(IB, inp["i_b"])
    pack(OB, inp["o_b"])
    pack(OLG, inp["oln_g"]); pack(OLB, inp["oln_b"])
    pvec[:, NEG2] = -2.0

    tags = inp["emb_table"].astype(np.float32)[np.asarray(inp["ent_ids"]).astype(np.int64)]  # [T, H]
    assert tags.shape[0] == 50, f"program compiled for 50 tags, got {tags.shape[0]}"
    tagsT = np.zeros((H, 64), F8)
    tagsT[:, :tags.shape[0]] = (tags.T * WS).astype(F8)
    ident = np.eye(P, dtype=F8)
    ones = np.ones((P, 1), BF)

    # band tiles in local (rotated) coords, per half: for half=1 the rotated
    # tail rows (k_rot >= S - QL) correspond to true keys k_rot - S.
    # NOTE: the +1 mask rides through exp(s/8 - 2) as exp((s+8)/8 - 2), so
    # the staged band value must be 8 (the exp scale) per masked element.
    nb = max(len(band_kq), 1)
    band_h = np.zeros((2, nb, P, 512), F8)
    if band_kq and er > 0:
        for i, (kt, qn) in enumerate(band_kq):
            k_rot = kt * P + np.arange(P)[:, None]
            q_rot = qn * 512 + np.arange(512)[None, :]
            d = k_rot - q_rot
            m0 = np.abs(d) <= er
            m1 = np.where(k_rot >= S_ - QL, np.abs(d - S_) <= er, m0)
            band_h[0, i] = (m0 * 8.0).astype(F8)
            band_h[1, i] = (m1 * 8.0).astype(F8)

    in_maps = []
    for c in range(8):
        b, half = divmod(c, 2)
        xt = x[b].T  # [H, S]
        rot = np.concatenate([xt[:, half * QL:], xt[:, :half * QL]], axis=1)
        in_maps.append({
            "xT": np.ascontiguousarray(rot).astype(F8),
            "xh": np.ascontiguousarray(rot[:, :QL]) + so_b_eff[:, None],
            "w_sq": wcast["sq_w"], "w_sk": wcast["sk_w"], "w_sv": wcast["sv_w"],
            "w_so": wcast["so_w"], "w_cq": wcast["cq_w"], "w_ck": wcast["ck_w"],
            "w_cv": wcast["cv_w"], "w_co": wcast["co_w"],
            "w_i": wcast["i_w"], "w_o": wcast["o_w"],
            "pvec": pvec, "tagsT": tagsT, "ident": ident, "ones": ones,
            "band": np.ascontiguousarray(band_h[half]),
        })
    return in_maps


def kernel(**inputs):
    inp = {k: np.asarray(v) for k, v in inputs.items()}
    x = inp["x"]
    B, S_, H_ = x.shape
    er = int(inp["ent_range"])
    nc, band_kq = _get_program(er)
    in_maps = build_in_maps(inp, band_kq, er)

    res = run_bass_kernel_spmd(nc, in_maps, core_ids=list(range(8)))
    out = np.empty((B, S_, H_), np.float32)
    for c in range(8):
        b, half = divmod(c, 2)
        out[b, half * QL:(half + 1) * QL, :] = res.results[c]["yT"].T
    return out
